# revision 31
# baseline (speedup 1.0000x reference)
"""Trainium2 Bass kernel for a dense transformer decoder block.

Problem: B=2, S=2048, H=2048, NH=16 (head_dim=128), FFN=8192, fp32.

Sharding (zero collectives): 8 cores = 2 batches x 4 query-chunks of 512
contiguous rows.  Every core redundantly computes LN1 + K/V projections for
its full batch, then attention / WO / LN2 / FFN for its own 512 rows.  The
output is disjoint across cores; the host concatenates shards.

v2 layout: feature-major on device.  LN1 is chunked over the sequence so its
vector work overlaps K-projection matmuls; `a` (LN1 output) stays resident in
SBUF for both K and V projections; the causal mask is added on the PE via an
identity-lhsT accumulation matmul so softmax is a pure mm->exp chain;
attention is software-pipelined across heads; `h` (post-WO residual) stays
resident in SBUF; the FFN runs in bf16 (same PE rate, half the DMA).
"""

import json

import numpy as np

import concourse.bass as bass
import concourse.bass2jax as bass2jax
import concourse.mybir as mybir
import concourse.tile as tile
from concourse.bass_utils import compile_bir_kernel as _orig_compile_bir_kernel
from concourse.bass_utils import run_bass_kernel_spmd

F32 = mybir.dt.float32
F32R = mybir.dt.float32r
BF16 = mybir.dt.bfloat16
AF = mybir.ActivationFunctionType
OP = mybir.AluOpType

B, S, H, NH, HD, FF = 2, 2048, 2048, 16, 128, 8192
P = 128
QR = 512            # query rows per core
HT = H // P         # 16 feature tiles
FT = FF // P        # 64 ffn tiles
EPS = 1e-5
NEG = -1e4

# ---------------------------------------------------------------------------
# Workaround for this container's walrus build: it supports only ONE sync
# wait per instruction.  Rewrite the BIR just before walrus: an instruction
# with N>1 waits gets N-1 same-engine NoOps inserted before it, each carrying
# one wait.
# ---------------------------------------------------------------------------


def _split_multiwaits(bir_bytes):
    bir = json.loads(bir_bytes)
    ctr = 0
    for fn in bir.get("functions", []):
        for blk in fn.get("blocks", []):
            new = []
            for inst in blk.get("instructions", []):
                si = inst.get("sync_info")
                waits = (si or {}).get("on_wait") or []
                if len(waits) > 1:
                    for w in waits[:-1]:
                        ctr += 1
                        new.append({
                            "engine": inst["engine"],
                            "ins": [],
                            "outs": [],
                            "name": f"I-mwsplit{ctr}",
                            "opcode": "NoOp",
                            "sync_info": {"on_update": [], "on_wait": [w]},
                            "text_hint": "multiwait_split",
                        })
                    si["on_wait"] = [waits[-1]]
                new.append(inst)
            blk["instructions"] = new
    return json.dumps(bir).encode()


def _patched_compile_bir_kernel(bir_json, tmpdir, neff_name="file.neff", **kw):
    if isinstance(bir_json, str):
        bir_json = bir_json.encode()
    return _orig_compile_bir_kernel(_split_multiwaits(bir_json), tmpdir,
                                    neff_name=neff_name, **kw)


def _install_patch():
    bass2jax.compile_bir_kernel = _patched_compile_bir_kernel


def r(ap):
    """View an fp32 AP as float32r (full-rate PE mode)."""
    return ap.bitcast(F32R)


# ---------------------------------------------------------------------------
# Device program
# ---------------------------------------------------------------------------


def build_nc():
    _install_patch()
    nc = bass.Bass("TRN2")

    xTq = nc.dram_tensor("xTq", (H, QR), F32, kind="ExternalInput")
    maskT = nc.dram_tensor("maskT", (S, QR), BF16, kind="ExternalInput")
    ident_d = nc.dram_tensor("ident_d", (P, P), BF16, kind="ExternalInput")
    ones_bf_d = nc.dram_tensor("ones_bf_d", (P, 1), BF16, kind="ExternalInput")
    ones_d = nc.dram_tensor("ones_d", (P, 1), F32, kind="ExternalInput")
    ones_r_d = nc.dram_tensor("ones_r_d", (1, P), F32, kind="ExternalInput")
    wq_t = nc.dram_tensor("wq_t", (HT, P, HT, P), BF16, kind="ExternalInput")
    wk_t = nc.dram_tensor("wk_t", (HT, P, HT, P), BF16, kind="ExternalInput")
    wv_r = nc.dram_tensor("wv_r", (P, HT, H), BF16, kind="ExternalInput")
    wo_t = nc.dram_tensor("wo_t", (HT, P, HT, P), BF16, kind="ExternalInput")
    w1_t = nc.dram_tensor("w1_t", (FT, P, HT, P), BF16, kind="ExternalInput")
    w2_t = nc.dram_tensor("w2_t", (HT, P, FT, P), BF16, kind="ExternalInput")
    bq = nc.dram_tensor("bq", (H,), F32, kind="ExternalInput")
    bk = nc.dram_tensor("bk", (H,), F32, kind="ExternalInput")
    bv = nc.dram_tensor("bv", (H,), F32, kind="ExternalInput")
    bwo = nc.dram_tensor("bwo", (H,), F32, kind="ExternalInput")
    b1 = nc.dram_tensor("b1", (FF,), F32, kind="ExternalInput")
    b2 = nc.dram_tensor("b2", (H,), F32, kind="ExternalInput")
    outT = nc.dram_tensor("outT", (H, QR), F32, kind="ExternalOutput")

    with tile.TileContext(nc) as tc:
        cm_const = tc.tile_pool(name="const", bufs=1)
        const = cm_const.__enter__()
        ones = const.tile([P, 1], F32, tag="ones")
        nc.sync.dma_start(r(ones[:]), r(ones_d[:]))
        ones_row = const.tile([1, P], F32, tag="ones_row")
        nc.sync.dma_start(r(ones_row[:]), r(ones_r_d[:]))
        ident = const.tile([P, P], BF16, tag="ident")
        nc.sync.dma_start(ident[:], ident_d[:])
        ones_bf = const.tile([P, 1], BF16, tag="ones_bf")
        nc.sync.dma_start(ones_bf[:], ones_bf_d[:])

        def bias_tile(name, dram_t, ntiles):
            t = const.tile([P, ntiles], F32, tag=f"b_{name}")
            nc.sync.dma_start(t[:], dram_t.rearrange("(t p) -> p t", p=P))
            return t

        bq_t = bias_tile("bq", bq, HT)
        bk_t = bias_tile("bk", bk, HT)
        bv_t = bias_tile("bv", bv, HT)
        bwo_t = bias_tile("bwo", bwo, HT)
        b1_t = bias_tile("b1", b1, FT)
        b2_t = bias_tile("b2", b2, HT)

        cm_dram = tc.tile_pool(name="dram", bufs=1, space="DRAM")
        dram = cm_dram.__enter__()
        k_own = dram.tile([H, QR], BF16, tag="k_own")
        v_own = [dram.tile([QR, H // 2], BF16, tag=f"v_own{i}",
                           name=f"v_own{i}") for i in range(2)]
        k_gath = dram.tile([4, H, QR], BF16, tag="k_gath")
        v_gath = [dram.tile([4, QR, H // 2], BF16, tag=f"v_gath{i}",
                            name=f"v_gath{i}") for i in range(2)]
        GROUPS = [[0, 1, 2, 3], [4, 5, 6, 7]]

        # q_res / av_res / h_res live across phases; h_res reuses the
        # q_res buffer (same tag, bufs=1) once attention has consumed q.
        cm_per = tc.tile_pool(name="persist", bufs=1)
        per_p = cm_per.__enter__()
        q_res = per_p.tile([P, HT, QR], BF16, tag="q", name="q_res")
        av_res = per_p.tile([P, HT, QR], BF16, tag="av", name="av_res")
        mask_res = per_p.tile([P, HT, QR], BF16, tag="mask_res")

        # ============ Phase A: LN1 (own rows) + K/Q/V proj + AllGather =====
        # Each core normalizes and projects only its own 512 rows; K and V
        # are then AllGathered across the 4-core batch group.  a_own holds
        # LN1(x_own) in bf16; psum accumulation stays fp32.
        a_own = None  # placeholder; allocated below

        def ln_chunk(lp, sps, bps, stage, dst3d, dst_sl, src_dram, src_sl, n):
            """LN a chunk of n cols: DMA f32 into `stage` [P,HT,n], compute
            stats, write the normalized result to dst3d[:, :, dst_sl] (bf16).
            """
            for i in range(HT):
                nc.sync.dma_start(r(stage[:, i, :]),
                                  r(src_dram[P * i:P * (i + 1), src_sl]))
            mean_ps = sps.tile([1, n], F32, tag="ln_mps", name="ln_mps")
            sq_ps = sps.tile([1, n], F32, tag="ln_sps", name="ln_sps")
            for i in range(HT):
                xsq = lp.tile([P, n], F32, tag="ln_sq", name="ln_sq")
                nc.scalar.activation(r(xsq[:]), stage[:, i, :], AF.Square)
                nc.tensor.matmul(mean_ps[:], r(ones[:]), r(stage[:, i, :]),
                                 start=(i == 0), stop=(i == HT - 1))
                nc.tensor.matmul(sq_ps[:], r(ones[:]), r(xsq[:]),
                                 start=(i == 0), stop=(i == HT - 1))
            mean = lp.tile([1, n], F32, tag="ln_mean", name="ln_mean")
            msq = lp.tile([1, n], F32, tag="ln_msq", name="ln_msq")
            rstd = lp.tile([1, n], F32, tag="ln_rstd", name="ln_rstd")
            nc.scalar.activation(r(mean[:]), mean_ps[:], AF.Copy, scale=1.0 / H)
            nc.scalar.activation(msq[:], sq_ps[:], AF.Copy, scale=1.0 / H)
            with nc.allow_low_precision(reason="f32r is fp32 bits"):
                nc.vector.tensor_mul(r(rstd[:]), mean[:], mean[:])
            nc.vector.tensor_sub(msq[:], msq[:], rstd[:])
            nc.vector.tensor_scalar_add(msq[:], msq[:], EPS)
            nc.vector.reciprocal(msq[:], msq[:])
            nc.scalar.activation(r(rstd[:]), msq[:], AF.Sqrt)
            bmean_ps = bps.tile([P, n], F32, tag="ln_bmps", name="ln_bmps")
            nc.tensor.matmul(bmean_ps[:], r(ones_row[:]), r(mean[:]),
                             start=True, stop=True)
            bmean = lp.tile([P, n], F32, tag="ln_bmean", name="ln_bmean")
            nc.scalar.activation(bmean[:], bmean_ps[:], AF.Copy)
            brstd_ps = bps.tile([P, n], F32, tag="ln_brps", name="ln_brps")
            nc.tensor.matmul(brstd_ps[:], r(ones_row[:]), r(rstd[:]),
                             start=True, stop=True)
            brstd = lp.tile([P, n], F32, tag="ln_brstd", name="ln_brstd")
            nc.scalar.activation(brstd[:], brstd_ps[:], AF.Copy)
            for i in range(HT):
                t1 = lp.tile([P, n], F32, tag="ln_t1", name="ln_t1")
                nc.vector.tensor_sub(t1[:], stage[:, i, :], bmean[:])
                nc.vector.tensor_mul(dst3d[:, i, dst_sl], t1[:], brstd[:])

        cm_wq = tc.tile_pool(name="wvres", bufs=1)
        wq_p = cm_wq.__enter__()
        wv_res = wq_p.tile([P, HT, H], BF16, tag="wv_res")
        cm_ares = tc.tile_pool(name="ares", bufs=1)
        arp = cm_ares.__enter__()
        a_own = arp.tile([P, HT, QR], BF16, tag="a_own")

        with tc.tile_pool(name="ln1", bufs=1) as lp, \
             tc.tile_pool(name="lnstage", bufs=1) as stp, \
             tc.tile_pool(name="ln1ps", bufs=1, space="PSUM") as lnps, \
             tc.tile_pool(name="kqproj", bufs=2) as kp, \
             tc.tile_pool(name="drains", bufs=16) as drp, \
             tc.tile_pool(name="kqps", bufs=2, space="PSUM") as kps:
            nc.sync.dma_start(wv_res[:], wv_r[:])
            nc.sync.dma_start(mask_res[:],
                              maskT.rearrange("(t p) s -> p t s", p=P))
            LC = 256
            for c in range(2):
                st = stp.tile([P, HT, LC], F32, tag="lnst", name="lnst")
                ln_chunk(lp, lnps, lnps, st, a_own,
                         slice(LC * c, LC * (c + 1)),
                         xTq, slice(LC * c, LC * (c + 1)), LC)
            # Q proj (own rows) -- streamed weights, before any gather
            for dM in range(HT):
                qw = kp.tile([P, HT, P], BF16, tag="kw", name=f"qw{dM}")
                nc.sync.dma_start(qw[:], wq_t[dM])
                ps = kps.tile([P, QR], F32, tag="kp", name="qp")
                for ht in range(HT):
                    nc.tensor.matmul(ps[:], qw[:, ht, :], a_own[:, ht, :],
                                     start=(ht == 0), stop=(ht == HT - 1))
                nc.scalar.activation(q_res[:, dM, :], ps[:], AF.Identity,
                                     bias=bq_t[:, dM:dM + 1])
            # K proj (own rows) -- streamed weights, then one gather
            for dM in range(HT):
                kw = kp.tile([P, HT, P], BF16, tag="kw", name=f"kw{dM}")
                nc.sync.dma_start(kw[:], wk_t[dM])
                ps = kps.tile([P, QR], F32, tag="kp", name="kp")
                for ht in range(HT):
                    nc.tensor.matmul(ps[:], kw[:, ht, :], a_own[:, ht, :],
                                     start=(ht == 0), stop=(ht == HT - 1))
                kst = drp.tile([P, QR], BF16, tag="kst", name="kst")
                nc.scalar.activation(kst[:], ps[:], AF.Identity,
                                     bias=bk_t[:, dM:dM + 1])
                nc.gpsimd.dma_start(k_own[P * dM:P * (dM + 1), :], kst[:])
            nc.gpsimd.collective_compute(
                "AllGather", OP.bypass, replica_groups=GROUPS,
                ins=[k_own[:]], outs=[k_gath[:]])
            # V proj (own rows) from resident weights -- packs under the
            # K gather; then one V gather
            for dc in range(4):
                if dc == 2:
                    nc.gpsimd.collective_compute(
                        "AllGather", OP.bypass, replica_groups=GROUPS,
                        ins=[v_own[0][:]], outs=[v_gath[0][:]])
                for sb in range(4):
                    ps = kps.tile([P, 512], F32, tag="kp", name="vp")
                    for ht in range(HT):
                        nc.tensor.matmul(
                            ps[:], a_own[:, ht, P * sb:P * (sb + 1)],
                            wv_res[:, ht, 512 * dc:512 * (dc + 1)],
                            start=(ht == 0), stop=(ht == HT - 1))
                    vst = drp.tile([P, 512], BF16, tag="vst", name="vst")
                    nc.scalar.activation(vst[:], ps[:], AF.Copy)
                    nc.gpsimd.dma_start(
                        v_own[dc // 2][P * sb:P * (sb + 1),
                                       512 * (dc % 2):512 * (dc % 2 + 1)],
                        vst[:])

            nc.gpsimd.collective_compute(
                "AllGather", OP.bypass, replica_groups=GROUPS,
                ins=[v_own[1][:]], outs=[v_gath[1][:]])
        cm_ares.__exit__(None, None, None)
        cm_wq.__exit__(None, None, None)
        cm_hp = tc.tile_pool(name="hpool", bufs=1)
        hp_p = cm_hp.__enter__()

        # ============ Phase B: attention (pipelined across heads) ==========
        with tc.tile_pool(name="attn", bufs=2) as ap_, \
             tc.tile_pool(name="attn1", bufs=1) as ap1, \
             tc.tile_pool(name="attnps", bufs=2, space="PSUM") as aps:
            kh_t = [None] * NH
            vh_t = [None] * NH
            pt_t = [None] * NH

            vh4_t = [None] * 4

            def load_head(h):
                kh_t[h] = ap_.tile([P, S], BF16, tag="kh", name=f"kh{h}")
                for j in range(4):
                    nc.sync.dma_start(kh_t[h][:, QR * j:QR * (j + 1)],
                                        k_gath[j][P * h:P * (h + 1), :])
                if h % 4 == 0:
                    g4 = h // 4
                    vh4_t[g4] = ap_.tile([P, HT, 512], BF16, tag="vh4",
                                         name=f"vh4_{g4}")
                    for j in range(4):
                        nc.sync.dma_start(
                            vh4_t[g4][:, 4 * j:4 * (j + 1), :],
                            v_gath[g4 // 2][j]
                            .rearrange("(t p) d -> p t d", p=P)
                            [:, :, 512 * (g4 % 2):512 * (g4 % 2 + 1)])
                vh_t[h] = vh4_t[h // 4]

            def scores_exp(h):
                pt_t[h] = ap_.tile([P, HT, QR], BF16, tag="pt", name=f"pt{h}")
                pt = pt_t[h]
                for kb in range(HT):
                    sp = aps.tile([P, QR], F32, tag="sp")
                    nc.tensor.matmul(sp[:], kh_t[h][:, P * kb:P * (kb + 1)],
                                     q_res[:, h, :], start=True, stop=False)
                    nc.tensor.matmul(sp[:], ident[:], mask_res[:, kb, :],
                                     start=False, stop=True)
                    nc.scalar.activation(pt[:, kb, :], sp[:], AF.Exp)
                kh_t[h] = None

            def denom_av(h):
                pt = pt_t[h]
                dn = aps.tile([1, QR], F32, tag="dn")
                for kb in range(HT):
                    nc.tensor.matmul(dn[:], ones_bf[:], pt[:, kb, :],
                                     start=(kb == 0), stop=(kb == HT - 1))
                rec = ap_.tile([1, QR], F32, tag="rec")
                with nc.allow_low_precision(reason="f32r is fp32 bits"):
                    nc.vector.reciprocal(r(rec[:]), dn[:])
                brec_ps = aps.tile([P, QR], F32, tag="brec_ps")
                nc.tensor.matmul(brec_ps[:], r(ones_row[:]), r(rec[:]),
                                 start=True, stop=True)
                brec = ap_.tile([P, QR], F32, tag="brec")
                nc.scalar.activation(brec[:], brec_ps[:], AF.Copy)
                avp = aps.tile([P, QR], F32, tag="avp")
                for kb in range(HT):
                    nc.tensor.matmul(avp[:],
                                     vh_t[h][:, kb,
                                             P * (h % 4):P * (h % 4 + 1)],
                                     pt[:, kb, :],
                                     start=(kb == 0), stop=(kb == HT - 1))
                avn = ap_.tile([P, QR], F32, tag="avn", name=f"avn{h}")
                nc.vector.tensor_mul(avn[:], avp[:], brec[:])
                nc.vector.tensor_scalar_add(av_res[:, h, :], avn[:],
                                            bv_t[:, h:h + 1])
                vh_t[h] = None
                pt_t[h] = None

            load_head(0)
            for h in range(NH):
                if h + 1 < NH:
                    load_head(h + 1)
                scores_exp(h)
                if h > 0:
                    denom_av(h - 1)
            denom_av(NH - 1)

        # ============ Phase C: WO + residual + LN2 (stats interleaved) =====
        h_res = hp_p.tile([P, HT, QR], F32, tag="h", name="h_res")
        cm_f = tc.tile_pool(name="fres", bufs=1)
        f_p = cm_f.__enter__()
        f_res = f_p.tile([P, FT, QR], BF16, tag="f_res")
        cm_g = tc.tile_pool(name="gres", bufs=1)
        g_p = cm_g.__enter__()
        g_res = g_p.tile([P, HT, QR], BF16, tag="g_res")
        with tc.tile_pool(name="wo", bufs=2) as wop, \
             tc.tile_pool(name="wops", bufs=2, space="PSUM") as wops, \
             tc.tile_pool(name="ln2", bufs=1) as l2p, \
             tc.tile_pool(name="ln2ps", bufs=1, space="PSUM") as l2ps:
            mean_ps = l2ps.tile([1, QR], F32, tag="ln2_mps")
            sq_ps = l2ps.tile([1, QR], F32, tag="ln2_sps")
            for dM in range(HT):
                wot = wop.tile([P, HT, P], BF16, tag="wot")
                nc.sync.dma_start(wot[:], wo_t[dM])
                xq_t = wop.tile([P, QR], F32, tag="xq_t")
                nc.sync.dma_start(xq_t[:], xTq[P * dM:P * (dM + 1), :])
                ps = wops.tile([P, QR], F32, tag="wopsum")
                for ht in range(HT):
                    nc.tensor.matmul(ps[:], wot[:, ht, :],
                                     av_res[:, ht, :],
                                     start=(ht == 0), stop=(ht == HT - 1))
                with nc.allow_low_precision(reason="f32r is fp32 bits"):
                    nc.vector.scalar_tensor_tensor(
                        r(h_res[:, dM, :]), ps[:], bwo_t[:, dM:dM + 1],
                        xq_t[:], op0=OP.add, op1=OP.add)
                xsq = l2p.tile([P, QR], F32, tag="ln2_sq", name="ln2_sq")
                nc.scalar.activation(r(xsq[:]), h_res[:, dM, :], AF.Square)
                nc.tensor.matmul(mean_ps[:], r(ones[:]), r(h_res[:, dM, :]),
                                 start=(dM == 0), stop=(dM == HT - 1))
                nc.tensor.matmul(sq_ps[:], r(ones[:]), r(xsq[:]),
                                 start=(dM == 0), stop=(dM == HT - 1))
            mean = l2p.tile([1, QR], F32, tag="ln2_mean")
            msq = l2p.tile([1, QR], F32, tag="ln2_msq")
            rstd = l2p.tile([1, QR], F32, tag="ln2_rstd")
            nc.scalar.activation(r(mean[:]), mean_ps[:], AF.Copy, scale=1.0 / H)
            nc.scalar.activation(msq[:], sq_ps[:], AF.Copy, scale=1.0 / H)
            with nc.allow_low_precision(reason="f32r is fp32 bits"):
                nc.vector.tensor_mul(r(rstd[:]), mean[:], mean[:])
            nc.vector.tensor_sub(msq[:], msq[:], rstd[:])
            nc.vector.tensor_scalar_add(msq[:], msq[:], EPS)
            nc.vector.reciprocal(msq[:], msq[:])
            nc.scalar.activation(r(rstd[:]), msq[:], AF.Sqrt)
            bmean_ps = wops.tile([P, QR], F32, tag="wopsum", name="l2bm")
            nc.tensor.matmul(bmean_ps[:], r(ones_row[:]), r(mean[:]),
                             start=True, stop=True)
            bmean = l2p.tile([P, QR], F32, tag="ln2_bmean")
            nc.scalar.activation(bmean[:], bmean_ps[:], AF.Copy)
            brstd_ps = wops.tile([P, QR], F32, tag="wopsum", name="l2br")
            nc.tensor.matmul(brstd_ps[:], r(ones_row[:]), r(rstd[:]),
                             start=True, stop=True)
            brstd = l2p.tile([P, QR], F32, tag="ln2_brstd")
            nc.scalar.activation(brstd[:], brstd_ps[:], AF.Copy)
            for i in range(HT):
                t1 = l2p.tile([P, QR], F32, tag="ln2_t1", name="ln2_t1")
                nc.vector.tensor_sub(t1[:], h_res[:, i, :], bmean[:])
                nc.vector.tensor_mul(g_res[:, i, :], t1[:], brstd[:])

        with tc.tile_pool(name="ffn1", bufs=3) as f1p, \
             tc.tile_pool(name="f1ps", bufs=2, space="PSUM") as f1ps:
            for fM in range(FT):
                w1t = f1p.tile([P, HT, P], BF16, tag="w1t")
                nc.sync.dma_start(w1t[:], w1_t[fM])
                ps = f1ps.tile([P, QR], F32, tag="f1psum")
                for ht in range(HT):
                    nc.tensor.matmul(ps[:], w1t[:, ht, :], g_res[:, ht, :],
                                     start=(ht == 0), stop=(ht == HT - 1))
                nc.scalar.activation(f_res[:, fM, :], ps[:], AF.Gelu,
                                     bias=b1_t[:, fM:fM + 1])
        cm_g.__exit__(None, None, None)

        with tc.tile_pool(name="ffn2", bufs=3) as f2p, \
             tc.tile_pool(name="f2ps", bufs=2, space="PSUM") as f2ps:
            for dM in range(HT):
                ps = f2ps.tile([P, QR], F32, tag="f2psum")
                for q4 in range(4):
                    w2t = f2p.tile([P, HT, P], BF16, tag="w2t")
                    nc.sync.dma_start(
                        w2t[:], w2_t[dM][:, 16 * q4:16 * (q4 + 1), :])
                    for ft in range(HT):
                        kk = 16 * q4 + ft
                        nc.tensor.matmul(ps[:], w2t[:, ft, :],
                                         f_res[:, kk, :],
                                         start=(kk == 0), stop=(kk == FT - 1))
                ost = f2p.tile([P, QR], F32, tag="ost")
                nc.vector.scalar_tensor_tensor(
                    ost[:], ps[:], b2_t[:, dM:dM + 1], h_res[:, dM, :],
                    op0=OP.add, op1=OP.add)
                nc.gpsimd.dma_start(outT[P * dM:P * (dM + 1), :], ost[:])
        cm_f.__exit__(None, None, None)
        cm_hp.__exit__(None, None, None)
        cm_per.__exit__(None, None, None)
        cm_dram.__exit__(None, None, None)
        cm_const.__exit__(None, None, None)

    return nc


# ---------------------------------------------------------------------------
# Host side
# ---------------------------------------------------------------------------

_CACHE = {}


def _get_nc():
    if "nc" not in _CACHE:
        _CACHE["nc"] = build_nc()
    return _CACHE["nc"]


def make_in_maps(inputs):
    import ml_dtypes
    BF = ml_dtypes.bfloat16
    x = np.asarray(inputs["x"], np.float32)
    scale = np.float32(1.0 / np.sqrt(HD))
    wqkv = np.asarray(inputs["wqkv_w"], np.float32)
    wqkv_b = np.asarray(inputs["wqkv_b"], np.float32)
    ln1w = np.asarray(inputs["ln1_w"], np.float32)
    ln1b = np.asarray(inputs["ln1_b"], np.float32)
    ln2w = np.asarray(inputs["ln2_w"], np.float32)
    ln2b = np.asarray(inputs["ln2_b"], np.float32)
    w1 = np.asarray(inputs["w1"], np.float32)
    wq, wk, wv = wqkv[:H], wqkv[H:2 * H], wqkv[2 * H:]

    def tile_kxm(wT, dt=np.float32):
        # [K, M] -> [mM, p, kt, m2] so each [128, kt*128] lhsT load is
        # contiguous per partition
        K_, M_ = wT.shape
        return np.ascontiguousarray(
            wT.reshape(K_ // P, P, M_ // P, P).transpose(2, 1, 0, 3)).astype(dt)

    # LN affine params are folded into the downstream projections:
    # W @ (y*w + b) = (W*diag(w)) @ y + W @ b   (exact algebra)
    shared = {
        "ones_d": np.ones((P, 1), np.float32),
        "ones_bf_d": np.ones((P, 1), BF),
        "ones_r_d": np.ones((1, P), np.float32),
        "ident_d": np.eye(P, dtype=np.float32).astype(BF),
        "wq_t": tile_kxm(wq.T * ln1w[:, None] * scale, BF),
        "wk_t": tile_kxm(wk.T * ln1w[:, None], BF),
        "wv_r": np.ascontiguousarray(
            (wv.T * ln1w[:, None]).reshape(HT, P, H)
            .transpose(1, 0, 2)).astype(BF),
        "wo_t": tile_kxm(np.asarray(inputs["wo_w"], np.float32).T, BF),
        "w1_t": tile_kxm(w1.T * ln2w[:, None], BF),
        "w2_t": tile_kxm(np.asarray(inputs["w2"], np.float32).T, BF),
        "bq": np.ascontiguousarray((wqkv_b[:H] + wq @ ln1b) * scale),
        "bk": np.ascontiguousarray(wqkv_b[H:2 * H] + wk @ ln1b),
        "bv": np.ascontiguousarray(wqkv_b[2 * H:] + wv @ ln1b),
        "bwo": np.asarray(inputs["wo_b"], np.float32),
        "b1": np.asarray(inputs["b1"], np.float32) + w1 @ ln2b,
        "b2": np.asarray(inputs["b2"], np.float32),
    }
    kidx = np.arange(S)
    in_maps = []
    for core in range(8):
        b, c = divmod(core, 4)
        q0 = QR * c
        qidx = q0 + np.arange(QR)
        m = np.where(kidx[:, None] <= qidx[None, :], np.float32(0),
                     np.float32(NEG)).astype(BF)
        in_maps.append(dict(
            shared,
            xTq=np.ascontiguousarray(x[b, q0:q0 + QR].T),
            maskT=np.ascontiguousarray(m),
        ))
    return in_maps


def run_cores(inputs, **run_kw):
    nc = _get_nc()
    in_maps = make_in_maps(inputs)
    return nc, run_bass_kernel_spmd(nc, in_maps, core_ids=list(range(8)),
                                    **run_kw)


def kernel(**inputs):
    _, res = run_cores(inputs)
    out = np.empty((B, S, H), np.float32)
    for core in range(8):
        b, c = divmod(core, 4)
        out[b, QR * c:QR * (c + 1), :] = res.results[core]["outT"].T
    return out


# revision 32
# speedup vs baseline: 1.0239x; 1.0239x over previous
"""Trainium2 Bass kernel for a dense transformer decoder block.

Problem: B=2, S=2048, H=2048, NH=16 (head_dim=128), FFN=8192, fp32.

Sharding (zero collectives): 8 cores = 2 batches x 4 query-chunks of 512
contiguous rows.  Every core redundantly computes LN1 + K/V projections for
its full batch, then attention / WO / LN2 / FFN for its own 512 rows.  The
output is disjoint across cores; the host concatenates shards.

v2 layout: feature-major on device.  LN1 is chunked over the sequence so its
vector work overlaps K-projection matmuls; `a` (LN1 output) stays resident in
SBUF for both K and V projections; the causal mask is added on the PE via an
identity-lhsT accumulation matmul so softmax is a pure mm->exp chain;
attention is software-pipelined across heads; `h` (post-WO residual) stays
resident in SBUF; the FFN runs in bf16 (same PE rate, half the DMA).
"""

import json

import numpy as np

import concourse.bass as bass
import concourse.bass2jax as bass2jax
import concourse.mybir as mybir
import concourse.tile as tile
from concourse.bass_utils import compile_bir_kernel as _orig_compile_bir_kernel
from concourse.bass_utils import run_bass_kernel_spmd

F32 = mybir.dt.float32
F32R = mybir.dt.float32r
BF16 = mybir.dt.bfloat16
AF = mybir.ActivationFunctionType
OP = mybir.AluOpType

B, S, H, NH, HD, FF = 2, 2048, 2048, 16, 128, 8192
P = 128
QR = 512            # query rows per core
HT = H // P         # 16 feature tiles
FT = FF // P        # 64 ffn tiles
EPS = 1e-5
NEG = -1e4

# ---------------------------------------------------------------------------
# Workaround for this container's walrus build: it supports only ONE sync
# wait per instruction.  Rewrite the BIR just before walrus: an instruction
# with N>1 waits gets N-1 same-engine NoOps inserted before it, each carrying
# one wait.
# ---------------------------------------------------------------------------


def _split_multiwaits(bir_bytes):
    bir = json.loads(bir_bytes)
    ctr = 0
    for fn in bir.get("functions", []):
        for blk in fn.get("blocks", []):
            new = []
            for inst in blk.get("instructions", []):
                si = inst.get("sync_info")
                waits = (si or {}).get("on_wait") or []
                if len(waits) > 1:
                    for w in waits[:-1]:
                        ctr += 1
                        new.append({
                            "engine": inst["engine"],
                            "ins": [],
                            "outs": [],
                            "name": f"I-mwsplit{ctr}",
                            "opcode": "NoOp",
                            "sync_info": {"on_update": [], "on_wait": [w]},
                            "text_hint": "multiwait_split",
                        })
                    si["on_wait"] = [waits[-1]]
                new.append(inst)
            blk["instructions"] = new
    return json.dumps(bir).encode()


def _patched_compile_bir_kernel(bir_json, tmpdir, neff_name="file.neff", **kw):
    if isinstance(bir_json, str):
        bir_json = bir_json.encode()
    return _orig_compile_bir_kernel(_split_multiwaits(bir_json), tmpdir,
                                    neff_name=neff_name, **kw)


def _install_patch():
    bass2jax.compile_bir_kernel = _patched_compile_bir_kernel


def r(ap):
    """View an fp32 AP as float32r (full-rate PE mode)."""
    return ap.bitcast(F32R)


# ---------------------------------------------------------------------------
# Device program
# ---------------------------------------------------------------------------


def build_nc():
    _install_patch()
    nc = bass.Bass("TRN2")

    xTq = nc.dram_tensor("xTq", (H, QR), F32, kind="ExternalInput")
    maskT = nc.dram_tensor("maskT", (S, QR), BF16, kind="ExternalInput")
    ident_d = nc.dram_tensor("ident_d", (P, P), BF16, kind="ExternalInput")
    ones_bf_d = nc.dram_tensor("ones_bf_d", (P, 1), BF16, kind="ExternalInput")
    ones_d = nc.dram_tensor("ones_d", (P, 1), F32, kind="ExternalInput")
    ones_r_d = nc.dram_tensor("ones_r_d", (1, P), F32, kind="ExternalInput")
    wq_t = nc.dram_tensor("wq_t", (HT, P, HT, P), BF16, kind="ExternalInput")
    wk_t = nc.dram_tensor("wk_t", (HT, P, HT, P), BF16, kind="ExternalInput")
    wv_r = nc.dram_tensor("wv_r", (P, HT, H), BF16, kind="ExternalInput")
    wo_t = nc.dram_tensor("wo_t", (HT, P, HT, P), BF16, kind="ExternalInput")
    w1_t = nc.dram_tensor("w1_t", (FT, P, HT, P), BF16, kind="ExternalInput")
    w2_t = nc.dram_tensor("w2_t", (HT, P, FT, P), BF16, kind="ExternalInput")
    bq = nc.dram_tensor("bq", (H,), F32, kind="ExternalInput")
    bk = nc.dram_tensor("bk", (H,), F32, kind="ExternalInput")
    bv = nc.dram_tensor("bv", (H,), F32, kind="ExternalInput")
    bwo = nc.dram_tensor("bwo", (H,), F32, kind="ExternalInput")
    b1 = nc.dram_tensor("b1", (FF,), F32, kind="ExternalInput")
    b2 = nc.dram_tensor("b2", (H,), F32, kind="ExternalInput")
    outT = nc.dram_tensor("outT", (H, QR), F32, kind="ExternalOutput")

    with tile.TileContext(nc) as tc:
        cm_const = tc.tile_pool(name="const", bufs=1)
        const = cm_const.__enter__()
        ones = const.tile([P, 1], F32, tag="ones")
        nc.sync.dma_start(r(ones[:]), r(ones_d[:]))
        ones_row = const.tile([1, P], F32, tag="ones_row")
        nc.sync.dma_start(r(ones_row[:]), r(ones_r_d[:]))
        ident = const.tile([P, P], BF16, tag="ident")
        nc.sync.dma_start(ident[:], ident_d[:])
        ones_bf = const.tile([P, 1], BF16, tag="ones_bf")
        nc.sync.dma_start(ones_bf[:], ones_bf_d[:])

        def bias_tile(name, dram_t, ntiles):
            t = const.tile([P, ntiles], F32, tag=f"b_{name}")
            nc.sync.dma_start(t[:], dram_t.rearrange("(t p) -> p t", p=P))
            return t

        bq_t = bias_tile("bq", bq, HT)
        bk_t = bias_tile("bk", bk, HT)
        bv_t = bias_tile("bv", bv, HT)
        bwo_t = bias_tile("bwo", bwo, HT)
        b1_t = bias_tile("b1", b1, FT)
        b2_t = bias_tile("b2", b2, HT)

        cm_dram = tc.tile_pool(name="dram", bufs=1, space="DRAM")
        dram = cm_dram.__enter__()
        k_own = dram.tile([H, QR], BF16, tag="k_own")
        v_own = [dram.tile([QR, H // 2], BF16, tag=f"v_own{i}",
                           name=f"v_own{i}") for i in range(2)]
        k_gath = dram.tile([4, H, QR], BF16, tag="k_gath")
        v_gath = [dram.tile([4, QR, H // 2], BF16, tag=f"v_gath{i}",
                            name=f"v_gath{i}") for i in range(2)]
        GROUPS = [[0, 1, 2, 3], [4, 5, 6, 7]]

        # q_res / av_res / h_res live across phases; h_res reuses the
        # q_res buffer (same tag, bufs=1) once attention has consumed q.
        cm_per = tc.tile_pool(name="persist", bufs=1)
        per_p = cm_per.__enter__()
        q_res = per_p.tile([P, HT, QR], BF16, tag="q", name="q_res")
        av_res = per_p.tile([P, HT, QR], BF16, tag="av", name="av_res")
        mask_res = per_p.tile([P, HT, QR], BF16, tag="mask_res")

        # ============ Phase A: LN1 (own rows) + K/Q/V proj + AllGather =====
        # Each core normalizes and projects only its own 512 rows; K and V
        # are then AllGathered across the 4-core batch group.  a_own holds
        # LN1(x_own) in bf16; psum accumulation stays fp32.
        a_own = None  # placeholder; allocated below

        def ln_chunk(lp, sps, bps, stage, dst3d, dst_sl, src_dram, src_sl, n):
            """LN a chunk of n cols: DMA f32 into `stage` [P,HT,n], compute
            stats, write the normalized result to dst3d[:, :, dst_sl] (bf16).
            """
            for i in range(HT):
                nc.sync.dma_start(r(stage[:, i, :]),
                                  r(src_dram[P * i:P * (i + 1), src_sl]))
            mean_ps = sps.tile([1, n], F32, tag="ln_mps", name="ln_mps")
            sq_ps = sps.tile([1, n], F32, tag="ln_sps", name="ln_sps")
            for i in range(HT):
                xsq = lp.tile([P, n], F32, tag="ln_sq", name="ln_sq")
                nc.scalar.activation(r(xsq[:]), stage[:, i, :], AF.Square)
                nc.tensor.matmul(mean_ps[:], r(ones[:]), r(stage[:, i, :]),
                                 start=(i == 0), stop=(i == HT - 1))
                nc.tensor.matmul(sq_ps[:], r(ones[:]), r(xsq[:]),
                                 start=(i == 0), stop=(i == HT - 1))
            mean = lp.tile([1, n], F32, tag="ln_mean", name="ln_mean")
            msq = lp.tile([1, n], F32, tag="ln_msq", name="ln_msq")
            rstd = lp.tile([1, n], F32, tag="ln_rstd", name="ln_rstd")
            nc.scalar.activation(r(mean[:]), mean_ps[:], AF.Copy, scale=1.0 / H)
            nc.scalar.activation(msq[:], sq_ps[:], AF.Copy, scale=1.0 / H)
            with nc.allow_low_precision(reason="f32r is fp32 bits"):
                nc.vector.tensor_mul(r(rstd[:]), mean[:], mean[:])
            nc.vector.tensor_sub(msq[:], msq[:], rstd[:])
            nc.vector.tensor_scalar_add(msq[:], msq[:], EPS)
            nc.vector.reciprocal(msq[:], msq[:])
            nc.scalar.activation(r(rstd[:]), msq[:], AF.Sqrt)
            bmean_ps = bps.tile([P, n], F32, tag="ln_bmps", name="ln_bmps")
            nc.tensor.matmul(bmean_ps[:], r(ones_row[:]), r(mean[:]),
                             start=True, stop=True)
            bmean = lp.tile([P, n], F32, tag="ln_bmean", name="ln_bmean")
            nc.scalar.activation(bmean[:], bmean_ps[:], AF.Copy)
            brstd_ps = bps.tile([P, n], F32, tag="ln_brps", name="ln_brps")
            nc.tensor.matmul(brstd_ps[:], r(ones_row[:]), r(rstd[:]),
                             start=True, stop=True)
            brstd = lp.tile([P, n], F32, tag="ln_brstd", name="ln_brstd")
            nc.scalar.activation(brstd[:], brstd_ps[:], AF.Copy)
            for i in range(HT):
                t1 = lp.tile([P, n], F32, tag="ln_t1", name="ln_t1")
                nc.vector.tensor_sub(t1[:], stage[:, i, :], bmean[:])
                nc.vector.tensor_mul(dst3d[:, i, dst_sl], t1[:], brstd[:])

        cm_wq = tc.tile_pool(name="wvres", bufs=1)
        wq_p = cm_wq.__enter__()
        wv_res = wq_p.tile([P, HT, H], BF16, tag="wv_res")
        cm_ares = tc.tile_pool(name="ares", bufs=1)
        arp = cm_ares.__enter__()
        a_own = arp.tile([P, HT, QR], BF16, tag="a_own")

        with tc.tile_pool(name="ln1", bufs=1) as lp, \
             tc.tile_pool(name="lnstage", bufs=1) as stp, \
             tc.tile_pool(name="ln1ps", bufs=1, space="PSUM") as lnps, \
             tc.tile_pool(name="kqproj", bufs=2) as kp, \
             tc.tile_pool(name="drains", bufs=16) as drp, \
             tc.tile_pool(name="kqps", bufs=2, space="PSUM") as kps:
            LC = 256
            for c in range(2):
                st = stp.tile([P, HT, LC], F32, tag="lnst", name="lnst")
                ln_chunk(lp, lnps, lnps, st, a_own,
                         slice(LC * c, LC * (c + 1)),
                         xTq, slice(LC * c, LC * (c + 1)), LC)
            # Q proj (own rows) -- streamed weights, before any gather
            for dM in range(HT):
                qw = kp.tile([P, HT, P], BF16, tag="kw", name=f"qw{dM}")
                nc.sync.dma_start(qw[:], wq_t[dM])
                ps = kps.tile([P, QR], F32, tag="kp", name="qp")
                for ht in range(HT):
                    nc.tensor.matmul(ps[:], qw[:, ht, :], a_own[:, ht, :],
                                     start=(ht == 0), stop=(ht == HT - 1))
                nc.scalar.activation(q_res[:, dM, :], ps[:], AF.Identity,
                                     bias=bq_t[:, dM:dM + 1])
            nc.sync.dma_start(mask_res[:],
                              maskT.rearrange("(t p) s -> p t s", p=P))
            nc.sync.dma_start(wv_res[:], wv_r[:])
            # K proj (own rows) -- streamed weights, then one gather
            for dM in range(HT):
                kw = kp.tile([P, HT, P], BF16, tag="kw", name=f"kw{dM}")
                nc.sync.dma_start(kw[:], wk_t[dM])
                ps = kps.tile([P, QR], F32, tag="kp", name="kp")
                for ht in range(HT):
                    nc.tensor.matmul(ps[:], kw[:, ht, :], a_own[:, ht, :],
                                     start=(ht == 0), stop=(ht == HT - 1))
                kst = drp.tile([P, QR], BF16, tag="kst", name="kst")
                nc.scalar.activation(kst[:], ps[:], AF.Identity,
                                     bias=bk_t[:, dM:dM + 1])
                nc.gpsimd.dma_start(k_own[P * dM:P * (dM + 1), :], kst[:])
            nc.gpsimd.collective_compute(
                "AllGather", OP.bypass, replica_groups=GROUPS,
                ins=[k_own[:]], outs=[k_gath[:]])
            # V proj (own rows) from resident weights -- packs under the
            # K gather; then one V gather
            for dc in range(4):
                if dc == 2:
                    nc.gpsimd.collective_compute(
                        "AllGather", OP.bypass, replica_groups=GROUPS,
                        ins=[v_own[0][:]], outs=[v_gath[0][:]])
                for sb in range(4):
                    ps = kps.tile([P, 512], F32, tag="kp", name="vp")
                    for ht in range(HT):
                        nc.tensor.matmul(
                            ps[:], a_own[:, ht, P * sb:P * (sb + 1)],
                            wv_res[:, ht, 512 * dc:512 * (dc + 1)],
                            start=(ht == 0), stop=(ht == HT - 1))
                    vst = drp.tile([P, 512], BF16, tag="vst", name="vst")
                    nc.scalar.activation(vst[:], ps[:], AF.Copy)
                    nc.gpsimd.dma_start(
                        v_own[dc // 2][P * sb:P * (sb + 1),
                                       512 * (dc % 2):512 * (dc % 2 + 1)],
                        vst[:])

            nc.gpsimd.collective_compute(
                "AllGather", OP.bypass, replica_groups=GROUPS,
                ins=[v_own[1][:]], outs=[v_gath[1][:]])
        cm_ares.__exit__(None, None, None)
        cm_wq.__exit__(None, None, None)
        cm_hp = tc.tile_pool(name="hpool", bufs=1)
        hp_p = cm_hp.__enter__()

        # ============ Phase B: attention (pipelined across heads) ==========
        with tc.tile_pool(name="attn", bufs=2) as ap_, \
             tc.tile_pool(name="attn1", bufs=1) as ap1, \
             tc.tile_pool(name="attnps", bufs=2, space="PSUM") as aps:
            kh_t = [None] * NH
            vh_t = [None] * NH
            pt_t = [None] * NH

            def load_head(h):
                kh_t[h] = ap_.tile([P, S], BF16, tag="kh", name=f"kh{h}")
                vh_t[h] = ap_.tile([P, HT, P], BF16, tag="vh", name=f"vh{h}")
                for j in range(4):
                    nc.sync.dma_start(kh_t[h][:, QR * j:QR * (j + 1)],
                                        k_gath[j][P * h:P * (h + 1), :])
                    nc.sync.dma_start(
                        vh_t[h][:, 4 * j:4 * (j + 1), :],
                        v_gath[h // 8][j].rearrange("(t p) d -> p t d", p=P)
                        [:, :, P * (h % 8):P * (h % 8 + 1)])

            def scores_exp(h):
                pt_t[h] = ap_.tile([P, HT, QR], BF16, tag="pt", name=f"pt{h}")
                pt = pt_t[h]
                for kb in range(HT):
                    sp = aps.tile([P, QR], F32, tag="sp")
                    nc.tensor.matmul(sp[:], kh_t[h][:, P * kb:P * (kb + 1)],
                                     q_res[:, h, :], start=True, stop=False)
                    nc.tensor.matmul(sp[:], ident[:], mask_res[:, kb, :],
                                     start=False, stop=True)
                    nc.scalar.activation(pt[:, kb, :], sp[:], AF.Exp)
                kh_t[h] = None

            def denom_av(h):
                pt = pt_t[h]
                dn = aps.tile([1, QR], F32, tag="dn")
                for kb in range(HT):
                    nc.tensor.matmul(dn[:], ones_bf[:], pt[:, kb, :],
                                     start=(kb == 0), stop=(kb == HT - 1))
                rec = ap_.tile([1, QR], F32, tag="rec")
                with nc.allow_low_precision(reason="f32r is fp32 bits"):
                    nc.vector.reciprocal(r(rec[:]), dn[:])
                brec_ps = aps.tile([P, QR], F32, tag="brec_ps")
                nc.tensor.matmul(brec_ps[:], r(ones_row[:]), r(rec[:]),
                                 start=True, stop=True)
                brec = ap_.tile([P, QR], F32, tag="brec")
                nc.scalar.activation(brec[:], brec_ps[:], AF.Copy)
                avp = aps.tile([P, QR], F32, tag="avp")
                for kb in range(HT):
                    nc.tensor.matmul(avp[:], vh_t[h][:, kb, :],
                                     pt[:, kb, :],
                                     start=(kb == 0), stop=(kb == HT - 1))
                avn = ap_.tile([P, QR], F32, tag="avn", name=f"avn{h}")
                nc.vector.tensor_mul(avn[:], avp[:], brec[:])
                nc.vector.tensor_scalar_add(av_res[:, h, :], avn[:],
                                            bv_t[:, h:h + 1])
                vh_t[h] = None
                pt_t[h] = None

            load_head(0)
            for h in range(NH):
                if h + 1 < NH:
                    load_head(h + 1)
                scores_exp(h)
                if h > 0:
                    denom_av(h - 1)
            denom_av(NH - 1)

        # ============ Phase C: WO + residual + LN2 (stats interleaved) =====
        h_res = hp_p.tile([P, HT, QR], F32, tag="h", name="h_res")
        cm_f = tc.tile_pool(name="fres", bufs=1)
        f_p = cm_f.__enter__()
        f_res = f_p.tile([P, FT, QR], BF16, tag="f_res")
        cm_g = tc.tile_pool(name="gres", bufs=1)
        g_p = cm_g.__enter__()
        g_res = g_p.tile([P, HT, QR], BF16, tag="g_res")
        with tc.tile_pool(name="wo", bufs=2) as wop, \
             tc.tile_pool(name="wops", bufs=2, space="PSUM") as wops, \
             tc.tile_pool(name="ln2", bufs=1) as l2p, \
             tc.tile_pool(name="ln2ps", bufs=1, space="PSUM") as l2ps:
            mean_ps = l2ps.tile([1, QR], F32, tag="ln2_mps")
            sq_ps = l2ps.tile([1, QR], F32, tag="ln2_sps")
            for dM in range(HT):
                wot = wop.tile([P, HT, P], BF16, tag="wot")
                nc.sync.dma_start(wot[:], wo_t[dM])
                xq_t = wop.tile([P, QR], F32, tag="xq_t")
                nc.sync.dma_start(xq_t[:], xTq[P * dM:P * (dM + 1), :])
                ps = wops.tile([P, QR], F32, tag="wopsum")
                for ht in range(HT):
                    nc.tensor.matmul(ps[:], wot[:, ht, :],
                                     av_res[:, ht, :],
                                     start=(ht == 0), stop=(ht == HT - 1))
                with nc.allow_low_precision(reason="f32r is fp32 bits"):
                    nc.vector.scalar_tensor_tensor(
                        r(h_res[:, dM, :]), ps[:], bwo_t[:, dM:dM + 1],
                        xq_t[:], op0=OP.add, op1=OP.add)
                xsq = l2p.tile([P, QR], F32, tag="ln2_sq", name="ln2_sq")
                nc.scalar.activation(r(xsq[:]), h_res[:, dM, :], AF.Square)
                nc.tensor.matmul(mean_ps[:], r(ones[:]), r(h_res[:, dM, :]),
                                 start=(dM == 0), stop=(dM == HT - 1))
                nc.tensor.matmul(sq_ps[:], r(ones[:]), r(xsq[:]),
                                 start=(dM == 0), stop=(dM == HT - 1))
            mean = l2p.tile([1, QR], F32, tag="ln2_mean")
            msq = l2p.tile([1, QR], F32, tag="ln2_msq")
            rstd = l2p.tile([1, QR], F32, tag="ln2_rstd")
            nc.scalar.activation(r(mean[:]), mean_ps[:], AF.Copy, scale=1.0 / H)
            nc.scalar.activation(msq[:], sq_ps[:], AF.Copy, scale=1.0 / H)
            with nc.allow_low_precision(reason="f32r is fp32 bits"):
                nc.vector.tensor_mul(r(rstd[:]), mean[:], mean[:])
            nc.vector.tensor_sub(msq[:], msq[:], rstd[:])
            nc.vector.tensor_scalar_add(msq[:], msq[:], EPS)
            nc.vector.reciprocal(msq[:], msq[:])
            nc.scalar.activation(r(rstd[:]), msq[:], AF.Sqrt)
            bmean_ps = wops.tile([P, QR], F32, tag="wopsum", name="l2bm")
            nc.tensor.matmul(bmean_ps[:], r(ones_row[:]), r(mean[:]),
                             start=True, stop=True)
            bmean = l2p.tile([P, QR], F32, tag="ln2_bmean")
            nc.scalar.activation(bmean[:], bmean_ps[:], AF.Copy)
            brstd_ps = wops.tile([P, QR], F32, tag="wopsum", name="l2br")
            nc.tensor.matmul(brstd_ps[:], r(ones_row[:]), r(rstd[:]),
                             start=True, stop=True)
            brstd = l2p.tile([P, QR], F32, tag="ln2_brstd")
            nc.scalar.activation(brstd[:], brstd_ps[:], AF.Copy)
            for i in range(HT):
                t1 = l2p.tile([P, QR], F32, tag="ln2_t1", name="ln2_t1")
                nc.vector.tensor_sub(t1[:], h_res[:, i, :], bmean[:])
                nc.vector.tensor_mul(g_res[:, i, :], t1[:], brstd[:])

        with tc.tile_pool(name="ffn1", bufs=3) as f1p, \
             tc.tile_pool(name="f1ps", bufs=2, space="PSUM") as f1ps:
            for fM in range(FT):
                w1t = f1p.tile([P, HT, P], BF16, tag="w1t")
                nc.sync.dma_start(w1t[:], w1_t[fM])
                ps = f1ps.tile([P, QR], F32, tag="f1psum")
                for ht in range(HT):
                    nc.tensor.matmul(ps[:], w1t[:, ht, :], g_res[:, ht, :],
                                     start=(ht == 0), stop=(ht == HT - 1))
                nc.scalar.activation(f_res[:, fM, :], ps[:], AF.Gelu,
                                     bias=b1_t[:, fM:fM + 1])
        cm_g.__exit__(None, None, None)

        with tc.tile_pool(name="ffn2", bufs=3) as f2p, \
             tc.tile_pool(name="f2ps", bufs=2, space="PSUM") as f2ps:
            for dM in range(HT):
                ps = f2ps.tile([P, QR], F32, tag="f2psum")
                for q4 in range(4):
                    w2t = f2p.tile([P, HT, P], BF16, tag="w2t")
                    nc.sync.dma_start(
                        w2t[:], w2_t[dM][:, 16 * q4:16 * (q4 + 1), :])
                    for ft in range(HT):
                        kk = 16 * q4 + ft
                        nc.tensor.matmul(ps[:], w2t[:, ft, :],
                                         f_res[:, kk, :],
                                         start=(kk == 0), stop=(kk == FT - 1))
                ost = f2p.tile([P, QR], F32, tag="ost")
                nc.vector.scalar_tensor_tensor(
                    ost[:], ps[:], b2_t[:, dM:dM + 1], h_res[:, dM, :],
                    op0=OP.add, op1=OP.add)
                nc.gpsimd.dma_start(outT[P * dM:P * (dM + 1), :], ost[:])
        cm_f.__exit__(None, None, None)
        cm_hp.__exit__(None, None, None)
        cm_per.__exit__(None, None, None)
        cm_dram.__exit__(None, None, None)
        cm_const.__exit__(None, None, None)

    return nc


# ---------------------------------------------------------------------------
# Host side
# ---------------------------------------------------------------------------

_CACHE = {}


def _get_nc():
    if "nc" not in _CACHE:
        _CACHE["nc"] = build_nc()
    return _CACHE["nc"]


def make_in_maps(inputs):
    import ml_dtypes
    BF = ml_dtypes.bfloat16
    x = np.asarray(inputs["x"], np.float32)
    scale = np.float32(1.0 / np.sqrt(HD))
    wqkv = np.asarray(inputs["wqkv_w"], np.float32)
    wqkv_b = np.asarray(inputs["wqkv_b"], np.float32)
    ln1w = np.asarray(inputs["ln1_w"], np.float32)
    ln1b = np.asarray(inputs["ln1_b"], np.float32)
    ln2w = np.asarray(inputs["ln2_w"], np.float32)
    ln2b = np.asarray(inputs["ln2_b"], np.float32)
    w1 = np.asarray(inputs["w1"], np.float32)
    wq, wk, wv = wqkv[:H], wqkv[H:2 * H], wqkv[2 * H:]

    def tile_kxm(wT, dt=np.float32):
        # [K, M] -> [mM, p, kt, m2] so each [128, kt*128] lhsT load is
        # contiguous per partition
        K_, M_ = wT.shape
        return np.ascontiguousarray(
            wT.reshape(K_ // P, P, M_ // P, P).transpose(2, 1, 0, 3)).astype(dt)

    # LN affine params are folded into the downstream projections:
    # W @ (y*w + b) = (W*diag(w)) @ y + W @ b   (exact algebra)
    shared = {
        "ones_d": np.ones((P, 1), np.float32),
        "ones_bf_d": np.ones((P, 1), BF),
        "ones_r_d": np.ones((1, P), np.float32),
        "ident_d": np.eye(P, dtype=np.float32).astype(BF),
        "wq_t": tile_kxm(wq.T * ln1w[:, None] * scale, BF),
        "wk_t": tile_kxm(wk.T * ln1w[:, None], BF),
        "wv_r": np.ascontiguousarray(
            (wv.T * ln1w[:, None]).reshape(HT, P, H)
            .transpose(1, 0, 2)).astype(BF),
        "wo_t": tile_kxm(np.asarray(inputs["wo_w"], np.float32).T, BF),
        "w1_t": tile_kxm(w1.T * ln2w[:, None], BF),
        "w2_t": tile_kxm(np.asarray(inputs["w2"], np.float32).T, BF),
        "bq": np.ascontiguousarray((wqkv_b[:H] + wq @ ln1b) * scale),
        "bk": np.ascontiguousarray(wqkv_b[H:2 * H] + wk @ ln1b),
        "bv": np.ascontiguousarray(wqkv_b[2 * H:] + wv @ ln1b),
        "bwo": np.asarray(inputs["wo_b"], np.float32),
        "b1": np.asarray(inputs["b1"], np.float32) + w1 @ ln2b,
        "b2": np.asarray(inputs["b2"], np.float32),
    }
    kidx = np.arange(S)
    in_maps = []
    for core in range(8):
        b, c = divmod(core, 4)
        q0 = QR * c
        qidx = q0 + np.arange(QR)
        m = np.where(kidx[:, None] <= qidx[None, :], np.float32(0),
                     np.float32(NEG)).astype(BF)
        in_maps.append(dict(
            shared,
            xTq=np.ascontiguousarray(x[b, q0:q0 + QR].T),
            maskT=np.ascontiguousarray(m),
        ))
    return in_maps


def run_cores(inputs, **run_kw):
    nc = _get_nc()
    in_maps = make_in_maps(inputs)
    return nc, run_bass_kernel_spmd(nc, in_maps, core_ids=list(range(8)),
                                    **run_kw)


def kernel(**inputs):
    _, res = run_cores(inputs)
    out = np.empty((B, S, H), np.float32)
    for core in range(8):
        b, c = divmod(core, 4)
        out[b, QR * c:QR * (c + 1), :] = res.results[core]["outT"].T
    return out
